# revision 39
# baseline (speedup 1.0000x reference)
"""Trainium2 Bass kernel for nn_CausalSelfAttention (B=2, T=2048, C=1024, 16 heads).

Sharding: 8 cores = 2 batches x 4 head-groups (4 heads each). Each core:
  - computes Q^T/K^T ([d,t] layout) and V ([t,d]) for its heads from x^T
    (host pre-transposes x and pre-slices the weights),
  - runs causal flash attention in S^T ([k,q]) space: softmax denominators
    come free from a ones-column in the P@V matmul; exp on ScalarE with the
    1/sqrt(d) scale fused; no max-subtraction (logits are O(1) for this
    problem's scale),
  - projects through its W_out row-slice producing a partial [T, C] output.
Host sums the 4 tensor-parallel partials per batch (the "all-reduce") and
adds b_out.

Matmul inputs are bf16 (fp32 accumulation in PSUM).
"""
import sys

if '/opt/trn_rl_repo' not in sys.path:
    sys.path.insert(0, '/opt/trn_rl_repo')

import numpy as np
import ml_dtypes

B, T, C = 2, 2048, 1024
N_HEAD = 16
D = 64
P = 128
N_CORES = 8
GROUPS = N_CORES // B            # 4 tensor-parallel groups per batch
HPC = N_HEAD // GROUPS           # 4 heads per core
DH = HPC * D                     # 256 head dims per core
KO = C // P                      # 8 contraction subtiles for projections
NT = T // P                      # 16 t tiles of 128
NQB = T // 512                   # 4 q blocks of 512
SCALE = 1.0 / np.sqrt(D)

_CACHE = {}


def _build():
    import concourse.mybir as mybir
    import concourse.tile as tile
    from concourse import bacc

    f32 = mybir.dt.float32
    bf16 = mybir.dt.bfloat16

    nc = bacc.Bacc("TRN2", target_bir_lowering=False, debug=False,
                   num_devices=N_CORES)

    xt_d = nc.dram_tensor("xt", [C, T], bf16, kind="ExternalInput")
    w_d = nc.dram_tensor("w", [C, 3 * DH], bf16, kind="ExternalInput")
    bqkv_d = nc.dram_tensor("bqkv", [3 * DH], f32, kind="ExternalInput")
    wo_d = nc.dram_tensor("wo", [DH, C], bf16, kind="ExternalInput")
    out_d = nc.dram_tensor("out", [T, C], mybir.dt.float16, kind="ExternalOutput")

    EXP = mybir.ActivationFunctionType.Exp
    IDEN = mybir.ActivationFunctionType.Identity

    with tile.TileContext(nc) as tc:
        with tc.tile_pool(name="persist", bufs=1) as pp:
            # xt split by T-quarters so matmuls can start early
            xts_t = [pp.tile([P, KO, 512], bf16, tag=f"xt{q}", name=f"xt{q}")
                     for q in range(4)]
            wv = pp.tile([P, KO, DH], bf16, tag="wv")
            wqk = pp.tile([P, KO, 2 * DH], bf16, tag="wqk")
            # per (head-pair, T-quarter) Q^T/K^T tiles: fine granularity so
            # attention S-matmuls become ready as soon as their slice exists
            qts = [[pp.tile([P, 512], bf16, tag=f"qt{s}_{q}", name=f"qt{s}_{q}")
                    for q in range(4)] for s in range(2)]
            kts = [[pp.tile([P, 512], bf16, tag=f"kt{s}_{q}", name=f"kt{s}_{q}")
                    for q in range(4)] for s in range(2)]
            vos = [pp.tile([P, 4, HPC, D + 1], bf16, tag=f"vo{q}",
                           name=f"vo{q}") for q in range(4)]
            # per q-block O^T tiles so the out-projection can start early
            ots = [[pp.tile([P, 512], bf16, tag=f"ot{j}_{hs}",
                            name=f"ot{j}_{hs}") for hs in range(2)]
                   for j in range(NQB)]
            wo = pp.tile([P, 2, C], bf16, tag="wo")
            bq = pp.tile([P, 2], f32, tag="bq")
            bk = pp.tile([P, 2], f32, tag="bk")
            bvb = pp.tile([P, DH], f32, tag="bvb")
            tri = pp.tile([P, P], bf16, tag="tri")

            # ---- loads & constants ----
            xt_r = xt_d.rearrange("(ko p) t -> p ko t", p=P)
            w_r = w_d.rearrange("(ko p) n -> p ko n", p=P)
            nc.sync.dma_start(wqk[:], w_r[:, :, 0:2 * DH])
            nc.sync.dma_start(xts_t[0][:], xt_r[:, :, 0:512])
            nc.sync.dma_start(xts_t[1][:], xt_r[:, :, 512:1024])
            nc.sync.dma_start(wv[:], w_r[:, :, 2 * DH:3 * DH])
            for q in range(2, 4):
                nc.sync.dma_start(xts_t[q][:],
                                  xt_r[:, :, 512 * q:512 * (q + 1)])
            nc.sync.dma_start(wo[:], wo_d.rearrange("(s p) c -> p s c", p=P))
            nc.sync.dma_start(bq[:], bqkv_d[0:DH].rearrange("(s p) -> p s", p=P))
            nc.sync.dma_start(bk[:], bqkv_d[DH:2 * DH].rearrange("(s p) -> p s", p=P))
            bvrow = pp.tile([1, DH], f32, tag="bvrow")
            nc.sync.dma_start(bvrow[0:1, :],
                              bqkv_d[2 * DH:3 * DH].rearrange("(o n) -> o n", o=1))
            nc.gpsimd.partition_broadcast(bvb[:, :], bvrow[0:1, :])
            # causal triangle mask: tri[k, q] = 1 if q >= k else 0
            nc.gpsimd.memset(tri[:], 1.0)
            nc.gpsimd.affine_select(
                out=tri[:], in_=tri[:], compare_op=mybir.AluOpType.is_ge,
                fill=0.0, base=0, pattern=[[1, P]], channel_multiplier=-1)
            for q in range(4):
                nc.vector.memset(vos[q][:, :, :, D:D + 1], 1.0)
            # trigger the exp ACT-table load early so phase B doesn't pay it
            scr = pp.tile([1, 1], f32, tag="scr")
            nc.scalar.activation(scr[0:1, 0:1], tri[0:1, 0:1], EXP)

            def xt_slice(ko, lo, hi):
                q = lo // 512
                assert hi <= 512 * (q + 1)
                return xts_t[q][:, ko, lo - 512 * q: hi - 512 * q]

            # ---- phases A+B interleaved: project quarter q, then run
            # attention q-block j=q (its Q slice and all needed K/V are ready)
            with (
                tc.tile_pool(name="psS", bufs=2, space="PSUM") as psS,
                tc.tile_pool(name="psO", bufs=2, space="PSUM") as psO,
                tc.tile_pool(name="wB", bufs=8) as wB,
            ):
                with (
                    tc.tile_pool(name="psA", bufs=1, space="PSUM") as psA,
                    tc.tile_pool(name="psV", bufs=1, space="PSUM") as psV,
                ):
                    def emit_qk(q):
                        for s_ in range(2):
                            for dst, wofs, bias in ((qts[s_][q], 0, bq),
                                                    (kts[s_][q], DH, bk)):
                                pq = psA.tile([P, 512], f32, tag="pq")
                                for ko in range(KO):
                                    nc.tensor.matmul(
                                        pq[:],
                                        wqk[:, ko,
                                            wofs + s_ * P: wofs + (s_ + 1) * P],
                                        xt_slice(ko, q * 512, (q + 1) * 512),
                                        start=(ko == 0), stop=(ko == KO - 1))
                                nc.vector.tensor_scalar_add(
                                    dst[:, :], pq[:], bias[:, s_:s_ + 1])

                    def emit_v(q):
                        for i in range(4 * q, 4 * q + 4):
                            pv = psV.tile([P, DH], f32, tag="pv")
                            for ko in range(KO):
                                nc.tensor.matmul(
                                    pv[:],
                                    xt_slice(ko, i * P, (i + 1) * P),
                                    wv[:, ko, :],
                                    start=(ko == 0), stop=(ko == KO - 1))
                            nc.vector.tensor_tensor(
                                vos[q][:, i % 4, :, 0:D],
                                pv.rearrange("p (h d) -> p h d", h=HPC),
                                bvb.rearrange("p (h d) -> p h d", h=HPC),
                                mybir.AluOpType.add)

                    def emit_attn(j, hs):
                        pls = [slice(0, D), slice(D, 2 * D)]
                        po_ts = [psO.tile([P, 512], f32, tag="po",
                                          name=f"po_{hs}_{j}_{u}")
                                 for u in range(2)]
                        nkt = 4 * (j + 1)
                        for i in range(nkt):
                            off = max(0, P * i - 512 * j)
                            width = 512 - off
                            sp = psS.tile([P, 2, 512], f32, tag="sp")
                            pt = wB.tile([P, 2, 512], bf16, tag="pt")
                            for u in range(2):
                                nc.tensor.matmul(
                                    sp[:, u, :width],
                                    kts[hs][i // 4][pls[u],
                                                    (i % 4) * P:
                                                    (i % 4 + 1) * P],
                                    qts[hs][j][pls[u], off:512],
                                    start=True, stop=True)
                            nc.scalar.activation(pt[:, :, :width],
                                                 sp[:, :, :width],
                                                 EXP, scale=float(SCALE))
                            for u in range(2):
                                if P * i >= 512 * j:  # diagonal triangle
                                    nc.vector.tensor_mul(
                                        pt[:, u, 0:P], pt[:, u, 0:P], tri[:])
                                nc.tensor.matmul(
                                    po_ts[u][0:D + 1, off:512],
                                    vos[i // 4][:, i % 4, 2 * hs + u, :],
                                    pt[:, u, :width],
                                    start=(i == 0), stop=(i == nkt - 1))
                        # normalize from an SBUF copy (frees PSUM early):
                        # row-move sums to partition 0, broadcast raw sums,
                        # reciprocal on the broadcast, multiply
                        for u in range(2):
                            ocp = wB.tile([P, 512], f32, tag="ocp")
                            nc.vector.tensor_copy(ocp[0:D + 1, :],
                                                  po_ts[u][0:D + 1, :])
                            r0 = wB.tile([1, 512], f32, tag="r0")
                            nc.sync.dma_start(r0[0:1, :], ocp[D:D + 1, :])
                            rb = wB.tile([D, 512], f32, tag="rb")
                            nc.gpsimd.partition_broadcast(rb[:, :], r0[0:1, :])
                            ri = wB.tile([D, 512], f32, tag="ri")
                            nc.vector.reciprocal_approx_fast(ri[:], rb[:])
                            ott = wB.tile([D, 512], bf16, tag="ott")
                            nc.vector.tensor_mul(ott[:], ocp[0:D, :], ri[:])
                            nc.sync.dma_start(ots[j][hs][pls[u], :], ott[:])

                    emit_qk(0)
                    emit_v(0)
                    for q in range(4):
                        emit_attn(q, 0)
                        if q < 3:
                            emit_qk(q + 1)
                        emit_attn(q, 1)
                        if q < 3:
                            emit_v(q + 1)

                # ---- phase C: output projection (interleaves with B via
                # per-j OT tiles) ----
                with (
                    tc.tile_pool(name="psC", bufs=2, space="PSUM") as psC,
                    tc.tile_pool(name="wC", bufs=8) as wC,
                ):
                    for jj in range(NQB):
                        for mo in range(4):
                            m = 4 * jj + mo
                            for n in range(2):
                                pc = psC.tile([P, 512], f32, tag="pc")
                                for s in range(2):
                                    nc.tensor.matmul(
                                        pc[:],
                                        ots[jj][s][:, mo * P:(mo + 1) * P],
                                        wo[:, s, n * 512:(n + 1) * 512],
                                        start=(s == 0), stop=(s == 1))
                                ob = wC.tile([P, 512], mybir.dt.float16, tag="ob")
                                if n == 0:
                                    nc.vector.tensor_copy(ob[:], pc[:])
                                else:
                                    nc.scalar.copy(ob[:], pc[:])
                                nc.sync.dma_start(
                                    out_d[m * P:(m + 1) * P,
                                          n * 512:(n + 1) * 512],
                                    ob[:])

    nc.compile()
    return nc


def _get_nc():
    if "nc" not in _CACHE:
        _CACHE["nc"] = _build()
    return _CACHE["nc"]


def _get_runner():
    """Build the jitted SPMD executor once (mirrors bass2jax.run_bass_via_pjrt
    but caches the jitted function so repeat calls skip retrace/recompile)."""
    if "runner" in _CACHE:
        return _CACHE["runner"]
    import jax
    import numpy as _np
    from jax.sharding import Mesh, PartitionSpec
    from jax.experimental.shard_map import shard_map
    import concourse.mybir as mybir
    from concourse import bass2jax

    nc = _get_nc()
    bass2jax.install_neuronx_cc_hook()

    partition_name = (nc.partition_id_tensor.name
                      if nc.partition_id_tensor else None)
    in_names, out_names, out_avals, zero_shapes = [], [], [], []
    for alloc in nc.m.functions[0].allocations:
        if not isinstance(alloc, mybir.MemoryLocationSet):
            continue
        name = alloc.memorylocations[0].name
        if alloc.kind == "ExternalInput":
            if name != partition_name:
                in_names.append(name)
        elif alloc.kind == "ExternalOutput":
            out_avals.append(jax.core.ShapedArray(
                tuple(alloc.tensor_shape), mybir.dt.np(alloc.dtype)))
            out_names.append(name)
            zero_shapes.append((tuple(alloc.tensor_shape),
                                mybir.dt.np(alloc.dtype)))
    n_params = len(in_names)
    n_outs = len(out_names)
    all_names = in_names + out_names
    if partition_name is not None:
        all_names = all_names + [partition_name]

    def _body(*args):
        operands = list(args)
        if partition_name is not None:
            operands.append(bass2jax.partition_id_tensor())
        outs = bass2jax._bass_exec_p.bind(
            *operands,
            out_avals=tuple(out_avals),
            in_names=tuple(all_names),
            out_names=tuple(out_names),
            lowering_input_output_aliases=(),
            sim_require_finite=True,
            sim_require_nnan=True,
            nc=nc,
        )
        return tuple(outs)

    devices = jax.devices()[:N_CORES]
    mesh = Mesh(_np.asarray(devices), ("core",))
    donate = tuple(range(n_params, n_params + n_outs))
    sharded = jax.jit(
        shard_map(_body, mesh=mesh,
                  in_specs=(PartitionSpec("core"),) * (n_params + n_outs),
                  out_specs=(PartitionSpec("core"),) * n_outs,
                  check_rep=False),
        donate_argnums=donate, keep_unused=True)

    def run(in_maps):
        concat_in = [
            _np.concatenate([_np.asarray(m[name]) for m in in_maps], axis=0)
            for name in in_names]
        concat_zeros = [
            _np.zeros((N_CORES * sh[0], *sh[1:]), dtype)
            for sh, dtype in zero_shapes]
        out_arrs = sharded(*concat_in, *concat_zeros)
        return [
            {name: _np.asarray(out_arrs[i]).reshape(
                N_CORES, *zero_shapes[i][0])[c]
             for i, name in enumerate(out_names)}
            for c in range(N_CORES)]

    _CACHE["runner"] = run
    return run


def kernel(x, mask, W_qkv, b_qkv, W_out, b_out):

    bf = ml_dtypes.bfloat16
    x = np.asarray(x, dtype=np.float32)
    W_qkv = np.asarray(W_qkv, dtype=np.float32)
    b_qkv = np.asarray(b_qkv, dtype=np.float32)
    W_out = np.asarray(W_out, dtype=np.float32)
    b_out = np.asarray(b_out, dtype=np.float32)
    # mask is the causal tril mask (per problem spec); causality is
    # implemented structurally on-device.

    run = _get_runner()

    xts = [np.ascontiguousarray(x[b].T).astype(bf) for b in range(B)]
    in_maps = []
    for core in range(N_CORES):
        b, g = divmod(core, GROUPS)
        cs = slice(g * DH, (g + 1) * DH)
        w_c = np.concatenate(
            [W_qkv[:, cs], W_qkv[:, C:][:, cs], W_qkv[:, 2 * C:][:, cs]],
            axis=1).astype(bf)
        bq_c = np.concatenate(
            [b_qkv[cs], b_qkv[C:][cs], b_qkv[2 * C:][cs]]).astype(np.float32)
        wo_c = np.ascontiguousarray(W_out[cs, :]).astype(bf)
        in_maps.append({"xt": xts[b], "w": np.ascontiguousarray(w_c),
                        "bqkv": bq_c, "wo": wo_c})

    results = run(in_maps)

    out = np.zeros((B, T, C), dtype=np.float32)
    for core in range(N_CORES):
        b = core // GROUPS
        out[b] += results[core]["out"].astype(np.float32)
    out += b_out[None, None, :]
    return out
